# revision 26
# baseline (speedup 1.0000x reference)
"""Causal self-attention on 8 Trainium2 NeuronCores.

Problem: B=2, T=2048, E=1024, H=16 heads (D=64), fp32.
  qkv = x @ W_qkv + b_qkv ; causal softmax attention ; y @ W_out + b_out

Sharding: core c handles batch b = c//4 and head group g = c%4 (4 heads,
256 of the 1024 hidden dims).  QKV + attention are computed fully locally
per core; per (q-tile, head-pair) the y_local.T rows are AllGather-ed
within each batch group of 4 cores, after which every core applies
W_out[:, own 256 cols] to the full y (Megatron-style column split).

Schedule (single in-order stream per engine, Tile-sync'd):
  for nt in 0..3:  # 512-wide t-window
    QK projections for window nt      (PE, K=128 matmuls)
    V projections for t-chunks of nt  (PE)
    attention q-tile j=nt, head pairs (scores K=64, row-tiled so the two
      heads of a pair run CONCURRENTLY on disjoint 64-row PE groups)
    per-pair AllGather of y_local.T   (overlapped with later compute)
    output projection of tile j-1     (PE, gathers long since done)

Softmax: transposed scores S.T[k, q]; exp on ScalarE (the only engine
with exp) straight out of PSUM; the V stationary carries 64 ones-columns
so the AV matmul materializes the denominator on PSUM partitions 64:128
-- a free PE broadcast that lets the reciprocal run 64-partitions-wide
on VectorE (reciprocal_approx_fast) instead of 1-wide.  Causal masking
is an additive -1e9 triangle on diagonal 128-blocks; fully-masked
columns are trimmed from matmul/exp/AV.

Inputs are pre-cast to bf16 on the host (layout prep); fp32 accumulation
in PSUM throughout.
"""

import os

import numpy as np

import concourse.bass as bass
import concourse.mybir as mybir
import concourse.tile as tile
from concourse import bacc
from concourse.bass_utils import run_bass_kernel_spmd

F32 = mybir.dt.float32
BF16 = mybir.dt.bfloat16
AF = mybir.ActivationFunctionType
OP = mybir.AluOpType

B, T, E, H = 2, 2048, 1024, 16
D = E // H            # 64 head dim
HL = 4                # heads per core
DL = HL * D           # 256 local hidden dims per core
NE = E // 128         # 8 contraction chunks
NT = T // 128         # 16 t-chunks
NJ = T // 512         # 4 q-tiles
SCALE = 1.0 / float(np.sqrt(D))
NEG = -1.0e9
RG = [[0, 1, 2, 3], [4, 5, 6, 7]]

# bisection toggles (defaults are the shipping configuration)
K_RECIP = os.environ.get("K_RECIP", "actln")     # actln | slow
K_ILV = os.environ.get("K_ILV", "0") == "1"      # interleave head-pair scores
# (measured: de-interleaved chains same-row-group matmuls back-to-back and
# avoids the ~170ns drain exposure on every row-group switch — 243.5us vs
# 245.5us interleaved)

_CACHE = {}


def _combined_exp_ln_tables(orig_fn):
    """Activation-table membership filter: the default first-match set
    choice puts Exp in `exp_and_others` and Ln in `natural_log`, reloading
    the ACT table (~1.3us) around every softmax-denominator reciprocal.
    Hiding Exp/Ln from the single-function sets makes the pass pick
    `natural_log_exp_and_others` (which genuinely contains both) so the
    whole kernel needs ONE table load.  Set ids/order are unchanged."""
    def patched(arch):
        out = {}
        for name, fns in orig_fn(arch).items():
            fns = set(fns)
            if name in ("exp_and_others", "exp_and_friends"):
                fns.discard(mybir.ActivationFunctionType.Exp)
            if name == "natural_log":
                fns.discard(mybir.ActivationFunctionType.Ln)
            out[name] = fns
        return out
    return patched


def build_bass():
    nc = bacc.Bacc("TRN2", target_bir_lowering=False, debug=False, num_devices=8)

    xT = nc.dram_tensor("xT", [E, T], BF16, kind="ExternalInput")
    wqkv = nc.dram_tensor("wqkv", [E, 3 * DL], BF16, kind="ExternalInput")
    bqkv = nc.dram_tensor("bqkv", [6, 128], F32, kind="ExternalInput")
    wout = nc.dram_tensor("wout", [E, DL], BF16, kind="ExternalInput")
    bout = nc.dram_tensor("bout", [2, 128], F32, kind="ExternalInput")
    out_shard = nc.dram_tensor("out_shard", [DL, T], F32, kind="ExternalOutput")

    # per-(q-tile, head-pair) staging for the pipelined AllGather
    ylocal = nc.dram_tensor("ylocal", [NJ, 2, 128, 512], BF16)
    ygat = nc.dram_tensor("ygat", [NJ, 2, 512, 512], BF16)

    with tile.TileContext(nc) as tc, \
         tc.tile_pool(name="const", bufs=1) as constp, \
         tc.tile_pool(name="sb", bufs=1) as sbp, \
         tc.tile_pool(name="exps", bufs=4) as expsp, \
         tc.tile_pool(name="attn", bufs=2) as attnp, \
         tc.tile_pool(name="out3", bufs=2) as out3, \
         tc.tile_pool(name="psS", bufs=2, space="PSUM") as psS, \
         tc.tile_pool(name="psO", bufs=2, space="PSUM") as psO, \
         tc.tile_pool(name="psX", bufs=2, space="PSUM") as psX:

        bq_s = constp.tile([128, 6], F32)
        nc.sync.dma_start(out=bq_s[:], in_=bqkv.ap().rearrange("m p -> p m"))
        bo_s = constp.tile([128, 2], F32)
        nc.sync.dma_start(out=bo_s[:], in_=bout.ap().rearrange("m p -> p m"))
        # force the ln+exp activation-table load off the critical path
        warm = constp.tile([1, 2], F32)
        nc.scalar.activation(warm[:, 0:1], bo_s[0:1, 0:1], AF.Exp)
        nc.scalar.activation(warm[:, 1:2], warm[:, 0:1], AF.Ln)
        # single [128, 128] additive triangle for the diagonal blocks
        tri = constp.tile([128, 128], F32)
        nc.gpsimd.memset(tri[:], 0.0)
        nc.gpsimd.affine_select(
            out=tri[:], in_=tri[:], compare_op=OP.is_ge, fill=NEG,
            base=0, pattern=[[1, 128]], channel_multiplier=-1)

        QT_s = sbp.tile([128, 2, T], BF16)
        KT_s = sbp.tile([128, 2, T], BF16)
        # V natural [t, vcol]; columns D:128 are ones so the AV matmul
        # broadcasts the softmax denominator onto PSUM partitions 64:128
        V_s = sbp.tile([128, NT, HL, 128], BF16)
        wo_s = sbp.tile([128, NE, DL], BF16)
        x_s = sbp.tile([128, NE, T], BF16)
        w_s = sbp.tile([128, NE, 3 * DL], BF16)

        nc.gpsimd.memset(V_s[:], 1.0)

        xT_r = xT.ap().rearrange("(c p) t -> p c t", p=128)
        wq_r = wqkv.ap().rearrange("(c p) m -> p c m", p=128)
        wo_r = wout.ap().rearrange("(c p) m -> p c m", p=128)
        # x streamed t-window-major so the first QK pass starts early
        for nt in range(NJ):
            for ec in range(NE):
                nc.gpsimd.dma_start(
                    out=x_s[:, ec, nt * 512:(nt + 1) * 512],
                    in_=xT_r[:, ec, nt * 512:(nt + 1) * 512])
        for ec in range(NE):
            nc.sync.dma_start(out=w_s[:, ec, :], in_=wq_r[:, ec, :])
        nc.sync.dma_start(out=wo_s[:], in_=wo_r)

        def emit_qk(nt):
            """Q^T/K^T for t-window nt: [128 qkv-cols, 512 t] per m-chunk."""
            win = slice(nt * 512, (nt + 1) * 512)
            p1 = psS.tile([128, 2, 512], F32, tag="psS", name=f"qkA{nt}")
            p2 = psS.tile([128, 2, 512], F32, tag="psS", name=f"qkB{nt}")
            for ec in range(NE):
                for ps_, m in ((p1, 2), (p1, 0), (p2, 3), (p2, 1)):
                    half = 0 if m >= 2 else 1
                    nc.tensor.matmul(
                        ps_[:, half, :],
                        w_s[:, ec, m * 128:(m + 1) * 128],
                        x_s[:, ec, win],
                        start=(ec == 0), stop=(ec == NE - 1))
            for ps_, half, dest, mm, m in (
                    (p1, 0, KT_s, 0, 2), (p1, 1, QT_s, 0, 0),
                    (p2, 0, KT_s, 1, 3), (p2, 1, QT_s, 1, 1)):
                # on DVE: ScalarE evacs queue behind the previous block's
                # exp backlog and delay the next block's scores (measured
                # +16us)
                nc.vector.tensor_scalar_add(
                    dest[:, mm, win], ps_[:, half, :], bq_s[:, m:m + 1])

        def emit_v(mtlo):
            """V natural [t, vcol] for 4 t-chunks."""
            for mt in range(mtlo, mtlo + 4):
                ps = psX.tile([128, 512], F32, tag="px", name=f"v{mt}")
                for ec in range(NE):
                    nc.tensor.matmul(
                        ps[:, 0:256],
                        x_s[:, ec, mt * 128:(mt + 1) * 128],
                        w_s[:, ec, 2 * DL:3 * DL],
                        start=(ec == 0), stop=(ec == NE - 1))
                nc.vector.tensor_copy(
                    V_s[:, mt, :, 0:D],
                    ps[:, 0:256].rearrange("p (a b) -> p a b", a=HL))

        def emit_av(j, hp, kp, eA, eB, poA, poB, nkc):
            for half in range(2):
                kc = 2 * kp + half
                off = max(0, 128 * kc - 512 * j)
                for po_, e_, h_ in ((poA, eA, 2 * hp), (poB, eB, 2 * hp + 1)):
                    nc.tensor.matmul(
                        po_[:, off:512], V_s[:, kc, h_, :],
                        e_[:, half, off:512],
                        start=(kc == 0), stop=(kc == nkc - 1))

        def attention_pair(j, hp, OTn):
            """Heads (2hp, 2hp+1): A on PE rows 0:64, B on rows 64:128 --
            interleaved score matmuls run concurrently (row tiling)."""
            nkc = 4 * j + 4
            poA = psO.tile([128, 512], F32, tag="po", name=f"poA{j}_{hp}")
            poB = psO.tile([128, 512], F32, tag="po", name=f"poB{j}_{hp}")
            prevA = prevB = None
            for kp in range(nkc // 2):
                psA = psS.tile([128, 2, 512], F32, tag="psS", name=f"sA{j}_{hp}_{kp}")
                psB = psS.tile([128, 2, 512], F32, tag="psS", name=f"sB{j}_{hp}_{kp}")
                eA = expsp.tile([128, 2, 512], BF16, tag="expS", name=f"eA{j}_{hp}_{kp}")
                eB = expsp.tile([128, 2, 512], BF16, tag="expS", name=f"eB{j}_{hp}_{kp}")
                offs = [max(0, 128 * (2 * kp + half) - 512 * j)
                        for half in range(2)]

                def emit_scores(ps_, rlo):
                    for half in range(2):
                        kc = 2 * kp + half
                        off = offs[half]
                        nc.tensor.matmul(
                            ps_[:, half, off:512],
                            KT_s[rlo:rlo + 64, hp, kc * 128:(kc + 1) * 128],
                            QT_s[rlo:rlo + 64, hp, j * 512 + off:(j + 1) * 512],
                            start=True, stop=True)

                if K_ILV:
                    # interleave A/B: disjoint 64-row PE groups run concurrently
                    for half in range(2):
                        kc = 2 * kp + half
                        off = offs[half]
                        for ps_, rlo in ((psA, 0), (psB, 64)):
                            nc.tensor.matmul(
                                ps_[:, half, off:512],
                                KT_s[rlo:rlo + 64, hp, kc * 128:(kc + 1) * 128],
                                QT_s[rlo:rlo + 64, hp,
                                     j * 512 + off:(j + 1) * 512],
                                start=True, stop=True)
                else:
                    emit_scores(psA, 0)
                    emit_scores(psB, 64)
                for half in range(2):
                    kc = 2 * kp + half
                    if 128 * kc >= 512 * j:
                        # diagonal block: additive triangle
                        off = offs[half]
                        for ps_ in (psA, psB):
                            nc.vector.tensor_tensor(
                                out=ps_[:, half, off:off + 128],
                                in0=ps_[:, half, off:off + 128],
                                in1=tri[:], op=OP.add)
                for ps_, e_ in ((psA, eA), (psB, eB)):
                    if offs == [0, 0]:
                        nc.scalar.activation(e_[:], ps_[:], AF.Exp, scale=SCALE)
                    else:
                        for half in range(2):
                            off = offs[half]
                            nc.scalar.activation(
                                e_[:, half, off:512], ps_[:, half, off:512],
                                AF.Exp, scale=SCALE)
                if kp > 0:
                    emit_av(j, hp, kp - 1, prevA, prevB, poA, poB, nkc)
                prevA, prevB = eA, eB
            emit_av(j, hp, nkc // 2 - 1, prevA, prevB, poA, poB, nkc)

            # normalize: po rows 64:128 all hold the denominator
            rA = attnp.tile([64, 512], F32, tag="recip", name=f"rA{j}_{hp}")
            rB = attnp.tile([64, 512], F32, tag="recip", name=f"rB{j}_{hp}")
            for r_, po_ in ((rA, poA), (rB, poB)):
                if K_RECIP == "actln":
                    # 1/d = exp(-ln d) on ScalarE: keeps the reciprocal off
                    # the DVE queue (where it would delay the next pair's
                    # causal-mask adds); ln+exp share one activation table.
                    nc.scalar.activation(r_[:], po_[64:128, :], AF.Ln)
                    nc.scalar.activation(r_[:], r_[:], AF.Exp, scale=-1.0)
                else:
                    nc.vector.reciprocal(r_[:], po_[64:128, :])
            nc.vector.tensor_tensor(
                out=OTn[0:64, hp, :], in0=poA[0:64, :], in1=rA[:], op=OP.mult)
            nc.vector.tensor_scalar_add(
                OTn[0:64, hp, :], OTn[0:64, hp, :], bq_s[0:64, 4 + hp:5 + hp])
            nc.vector.tensor_tensor(
                out=OTn[64:128, hp, :], in0=poB[0:64, :], in1=rB[:], op=OP.mult)
            nc.vector.tensor_scalar_add(
                OTn[64:128, hp, :], OTn[64:128, hp, :], bq_s[64:128, 4 + hp:5 + hp])
            # all-gather this (q-tile, head-pair) within the batch group
            nc.sync.dma_start(out=ylocal[j, hp], in_=OTn[:, hp, :])
            nc.gpsimd.collective_compute(
                "AllGather", OP.bypass, replica_groups=RG,
                ins=[ylocal[j, hp]], outs=[ygat[j, hp]])

        ytjs = {}

        def fetch_ytj_half(j, c2):
            """Assemble gathered y.T rows for pair c2 into SBUF,
            ec = 2*core + pair."""
            if j not in ytjs:
                ytjs[j] = out3.tile([128, NE, 512], BF16, tag="ytj",
                                    name=f"ytj{j}")
            ytj = ytjs[j]
            dst = bass.AP(
                ytj.tensor, ytj[:].offset + c2 * 512,
                [list(p) for p in ytj[:].ap[:1]] + [[1024, 4], [1, 512]])
            nc.sync.dma_start(
                out=dst, in_=ygat[j, c2].rearrange("(r p) t -> p r t", p=128))

        def emit_outproj(jj, part=None):
            """part=None: full;  part=(0|1): only ec chunks of that pair,
            so half the contraction can run while the other gather is in
            flight (used for the last q-tile's tail).  Each part is a
            closed accumulation group; part 1 continues onto part 0's
            PSUM via has_written (start=False, group check skipped)."""
            ytj = ytjs[jj]
            ecs = list(range(NE)) if part is None else list(range(part, NE, 2))
            for mc in range(2):
                if part is None or part == 0:
                    ytjs[("ps", jj, mc)] = psX.tile(
                        [128, 512], F32, tag="px", name=f"op{jj}_{mc}")
                ps = ytjs[("ps", jj, mc)]
                for i, ec in enumerate(ecs):
                    nc.tensor.matmul(
                        ps[:], wo_s[:, ec, mc * 128:(mc + 1) * 128],
                        ytj[:, ec, :],
                        start=(part != 1 and i == 0),
                        stop=(i == len(ecs) - 1),
                        skip_group_check=(part == 1))
                if part == 0:
                    continue
                ot = out3.tile([128, 512], F32, tag="ot", name=f"ot{jj}_{mc}")
                nc.vector.tensor_scalar_add(ot[:], ps[:], bo_s[:, mc:mc + 1])
                nc.sync.dma_start(
                    out=out_shard[mc * 128:(mc + 1) * 128,
                                  jj * 512:(jj + 1) * 512],
                    in_=ot[:])

        for nt in range(NJ):
            emit_qk(nt)
            emit_v(4 * nt)
            if nt > 0:
                emit_outproj(nt - 1)
            OTn = attnp.tile([128, 2, 512], BF16, tag="otn", name=f"otn{nt}")
            attention_pair(nt, 0, OTn)
            fetch_ytj_half(nt, 0)
            attention_pair(nt, 1, OTn)
            if nt == NJ - 1:
                # half the last output projection (pair-0 rows, gathered
                # during pair 1's attention) runs while pair 1's gather
                # is in flight
                emit_outproj(nt, part=0)
            fetch_ytj_half(nt, 1)
        emit_outproj(NJ - 1, part=1)

    import concourse.bacc as bacc_mod
    orig_tables = bacc_mod.get_activation_tables
    bacc_mod.get_activation_tables = _combined_exp_ln_tables(orig_tables)
    try:
        nc.compile()
    finally:
        bacc_mod.get_activation_tables = orig_tables
    return nc


def _get_nc():
    if "nc" not in _CACHE:
        _CACHE["nc"] = build_bass()
    return _CACHE["nc"]


def kernel(x, W_qkv, b_qkv, W_out, b_out, **run_kwargs):
    import ml_dtypes
    bf = ml_dtypes.bfloat16

    x = np.asarray(x, np.float32)
    W_qkv = np.asarray(W_qkv, np.float32)
    b_qkv = np.asarray(b_qkv, np.float32)
    W_out = np.asarray(W_out, np.float32)
    b_out = np.asarray(b_out, np.float32)

    in_maps = []
    for c in range(8):
        b, g = divmod(c, 4)
        cols = slice(g * DL, (g + 1) * DL)
        wq = W_qkv[:, 0 * E:1 * E][:, cols]
        wk = W_qkv[:, 1 * E:2 * E][:, cols]
        wv = W_qkv[:, 2 * E:3 * E][:, cols]
        bq = b_qkv[0 * E:1 * E][cols]
        bk = b_qkv[1 * E:2 * E][cols]
        bv = b_qkv[2 * E:3 * E][cols]
        in_maps.append({
            "xT": np.ascontiguousarray(x[b].T.astype(bf)),
            "wqkv": np.ascontiguousarray(
                np.concatenate([wq, wk, wv], axis=1).astype(bf)),
            "bqkv": np.concatenate([bq, bk, bv]).reshape(6, 128),
            "wout": np.ascontiguousarray(W_out[:, cols].astype(bf)),
            "bout": np.ascontiguousarray(b_out[cols]).reshape(2, 128),
        })

    res = run_bass_kernel_spmd(_get_nc(), in_maps, list(range(8)), **run_kwargs)
    _CACHE["last_results"] = res

    out = np.empty((B, T, E), np.float32)
    for c in range(8):
        b, g = divmod(c, 4)
        out[b][:, g * DL:(g + 1) * DL] = res.results[c]["out_shard"].T
    return out
